# revision 4
# baseline (speedup 1.0000x reference)
"""Trainium2 Bass kernel: gradient of the EnergyAttention scalar energy.

reference:
    q = einsum('bqd,hzd->bqhz', g, wq); k = einsum('bkd,hzd->bkhz', g, wk)
    scores = einsum('bqhz,bkhz->bhqk', q, k)
    E = -(logsumexp(BETA*scores, -1)/BETA).sum() + POS_SCALE*(g*pos).sum()
    out = dE/dg

Math: with P = softmax(BETA*scores) per (b,h,q):
    out[b] = -sum_h [ (P@K) @ wq_h + (P.T@Qn) @ wk_h ] + POS_SCALE*pos
where Qn = diag(1/Z) Q (row-normalized by the softmax partition Z).

Sharding: 8 cores; core c handles batch b=c//4 and heads 4*(c%4)..4*(c%4)+3
(two head-pairs packed into the 128-partition dim).  Each core emits the
positive partial 16*sum_h[(dQ)wq + (dK)wk] of shape [S, D] in fp16; the
host combines (scale, negate, sum over head-groups, add positional term).

v2 layout/dtype plan (per core, per head-pair):
  inputs:  gt = G^T fp16 [d, s], weights fp16
  proj:    QT2/KT2 [z2, s] fp16 matmuls (d contracted in 8 tiles)
  fp8:     q8/k8 = fp8(16*QT2/KT2), DMA-remapped to z-half layout
           qh/kh [32, (zhalf, s)] so scores run as fp8 DoubleRow
           matmuls (0.5 cycles/row: two 32-partition z-groups per pass)
  trans:   Qraw16/K2n16 [s, z2] fp16 = 16 * transpose(QT2/KT2)
  loop i:  scores/scoresT blocks via DoubleRow -> exp on ACT (fp16 out,
           fused row-sum accum for Z) -> dK^T += (16Q/Z)^T-block @ P-block,
           dQ^T += 16K-block @ PT-block
  out:     gout = sum_pairs dQT^T wq + dKT^T wk  (fp16, scaled x16)
"""

import numpy as np

B = 2
S = 1024
D = 1024
NH = 16
Z = 64
BETA = 1.0 / np.sqrt(np.float32(Z))
POS_SCALE = 0.001
N_CORES = 8
HPC = 4           # heads per core
NPAIR = 2         # head pairs per core
ND = D // 128     # 8 d-tiles
NQ = S // 128     # 8 q/k blocks
NCH = S // 512    # 2 moving-dim chunks
SC_Q = 16.0       # fp8 q/k prescale (avoids e4m3 subnormals)
SC_G = 16.0       # q2n/k2n prescale (gout comes out x SC_G)

_CACHE = {}


def build_nc(reps=1):
    """Build the (SPMD, identical-per-core) Bass program.

    reps>1 repeats the whole computation (idempotent) inside one NEFF --
    used to measure steady-state per-execution time as a marginal cost."""
    from contextlib import ExitStack

    import concourse.mybir as mybir
    import concourse.tile as tile
    from concourse import bacc
    from concourse.masks import make_identity

    F32 = mybir.dt.float32
    F16 = mybir.dt.float16
    F8 = mybir.dt.float8e4
    MUL = mybir.AluOpType.mult
    EXP = mybir.ActivationFunctionType.Exp
    DR = mybir.MatmulPerfMode.DoubleRow

    nc = bacc.Bacc(
        "TRN2",
        target_bir_lowering=False,
        debug=False,
        enable_asserts=False,
        num_devices=N_CORES,
    )

    xT = nc.dram_tensor("xT", [D, S], F16, kind="ExternalInput").ap()
    wqT2 = nc.dram_tensor("wqT2", [NPAIR * D, 128], F16, kind="ExternalInput").ap()
    wkT2 = nc.dram_tensor("wkT2", [NPAIR * D, 128], F16, kind="ExternalInput").ap()
    wq2n = nc.dram_tensor("wq2n", [NPAIR * 128, D], F16, kind="ExternalInput").ap()
    wk2n = nc.dram_tensor("wk2n", [NPAIR * 128, D], F16, kind="ExternalInput").ap()
    gout = nc.dram_tensor("gout", [S, D], F16, kind="ExternalOutput").ap()

    with tile.TileContext(nc) as tc, ExitStack() as ctx:
        sb1 = ctx.enter_context(tc.tile_pool(name="sb1", bufs=1))
        sb2 = ctx.enter_context(tc.tile_pool(name="sb2", bufs=2))
        sb4 = ctx.enter_context(tc.tile_pool(name="sb4", bufs=4))
        pp = ctx.enter_context(tc.tile_pool(name="pp", bufs=8))
        # PSUM: "sc" 2x[128,1024]f32 (4 banks) shared by proj/scores/
        # scoresT/outproj; fp16 transpose tiles are 1 bank each; "d"
        # 2x[128,1024]f32 (4 banks) for the dK then dQ accumulators.
        ps_sc = ctx.enter_context(tc.tile_pool(name="ps_sc", bufs=2, space="PSUM"))
        ps_d = ctx.enter_context(tc.tile_pool(name="ps_d", bufs=2, space="PSUM"))

        ident = sb1.tile([128, 128], F32, tag="ident")
        make_identity(nc, ident[:])
        ident_h = sb1.tile([128, 128], F16, tag="ident_h")
        nc.vector.tensor_copy(ident_h[:], ident[:])

        for _rep in range(reps):
            # ---- input loads (ordered so pair-0 proj can start ASAP) -----
            gt = sb1.tile([128, ND * S], F16, tag="gt")  # G^T: [d_in_tile, (dt, s)]
            wtq = sb1.tile([128, NPAIR * ND * 128], F16, tag="wtq")  # [d, (pair,dt,z2)]
            wtk = sb1.tile([128, NPAIR * ND * 128], F16, tag="wtk")
            nc.sync.dma_start(
                wtq[:, 0 : ND * 128].rearrange("p (b c) -> p b c", b=ND),
                wqT2[0:D, :].rearrange("(b p) c -> p b c", p=128),
            )
            for dt in range(ND):
                nc.sync.dma_start(
                    gt[:, dt * S : (dt + 1) * S], xT[dt * 128 : (dt + 1) * 128, :]
                )
                if dt == 0:
                    nc.sync.dma_start(
                        wtk[:, 0 : ND * 128].rearrange("p (b c) -> p b c", b=ND),
                        wkT2[0:D, :].rearrange("(b p) c -> p b c", p=128),
                    )

            nc.sync.dma_start(
                wtq[:, ND * 128 :].rearrange("p (b c) -> p b c", b=ND),
                wqT2[D:, :].rearrange("(b p) c -> p b c", p=128),
            )
            nc.sync.dma_start(
                wtk[:, ND * 128 :].rearrange("p (b c) -> p b c", b=ND),
                wkT2[D:, :].rearrange("(b p) c -> p b c", p=128),
            )
            wnq = sb1.tile([128, NPAIR * D], F16, tag="wnq")  # [z2, (pair, d)]
            wnk = sb1.tile([128, NPAIR * D], F16, tag="wnk")
            for p in range(NPAIR):
                nc.sync.dma_start(wnq[:, p * D : (p + 1) * D], wq2n[p * 128 : (p + 1) * 128, :])
                nc.sync.dma_start(wnk[:, p * D : (p + 1) * D], wk2n[p * 128 : (p + 1) * 128, :])

            # persistent across pairs
            dqt2 = sb1.tile([128, NPAIR * S], F16, tag="dqt2")  # [z2, (pair, q)]
            dkt2 = sb1.tile([128, NPAIR * S], F16, tag="dkt2")  # [z2, (pair, k)]
            zrowA = sb1.tile([1, S], F32, tag="zrowA")
            zrowB = sb1.tile([1, S], F32, tag="zrowB")
            ztsb = sb1.tile([16, 128], F32, tag="ztsb")

            pending_dq = []

            def emit_dq_burst():
                """dQ^T(unnorm) += 16K_i^T PT_i over all blocks, then Z-scale."""
                if not pending_dq:
                    return
                PT_a, k2n_a, zbc_ab, pa = pending_dq.pop()
                dq_ps = [
                    ps_d.tile([128, S], F32, tag="ps_d", name=f"dq_ps{pa}_{a}")
                    for a in range(2)
                ]
                for i in range(NQ):
                    for a in range(2):
                        for ch in range(NCH):
                            nc.tensor.matmul(
                                dq_ps[a][a * 64 : (a + 1) * 64, ch * 512 : (ch + 1) * 512],
                                lhsT=k2n_a[:, i * 128 + a * 64 : i * 128 + (a + 1) * 64],
                                rhs=PT_a[:, (a * NQ + i) * S + ch * 512 : (a * NQ + i) * S + ch * 512 + 512],
                                start=(i == 0),
                                stop=(i == NQ - 1),
                            )
                for a in range(2):
                    nc.vector.tensor_tensor(
                        dqt2[a * 64 : (a + 1) * 64, pa * S : (pa + 1) * S],
                        dq_ps[a][a * 64 : (a + 1) * 64, :],
                        zbc_ab[a][a * 64 : (a + 1) * 64, :],
                        MUL,
                    )

            for p in range(NPAIR):
                # ---- projections: QT2/KT2 [z2, s] fp16 + fp8 z-half copies --
                qt2 = sb2.tile([128, S], F16, tag="qt2")
                kt2 = sb2.tile([128, S], F16, tag="kt2")
                qh = sb2.tile([128, 2 * S], F8, tag="qh")  # [z32, (zhalf, s)] x2 heads
                kh = sb2.tile([128, 2 * S], F8, tag="kh")
                for wt, dst, dst8, dsth in ((wtq, qt2, None, qh), (wtk, kt2, None, kh)):
                    ps = ps_sc.tile([128, S], F32, tag="ps_sc", name=f"pj{p}_{dst.tensor.name}")
                    for dt in range(ND):
                        j = p * ND + dt
                        for ch in range(NCH):
                            nc.tensor.matmul(
                                ps[:, ch * 512 : (ch + 1) * 512],
                                lhsT=wt[:, j * 128 : (j + 1) * 128],
                                rhs=gt[:, dt * S + ch * 512 : dt * S + ch * 512 + 512],
                                start=(dt == 0),
                                stop=(dt == ND - 1),
                            )
                    t8 = sb2.tile([128, S], F8, tag=f"t8{dst.tensor.name[:1]}", name=f"t8{p}_{dst.tensor.name}")
                    for ch in range(NCH):
                        nc.vector.tensor_copy(
                            dst[:, ch * 512 : (ch + 1) * 512],
                            ps[:, ch * 512 : (ch + 1) * 512],
                        )
                        nc.vector.tensor_scalar(
                            t8[:, ch * 512 : (ch + 1) * 512],
                            ps[:, ch * 512 : (ch + 1) * 512],
                            float(SC_Q),
                            None,
                            MUL,
                        )
                    # z-half remap: [z2, s] -> [z32, (zhalf per head, s)]
                    for a in range(2):
                        for zh in range(2):
                            nc.sync.dma_start(
                                dsth[a * 32 : (a + 1) * 32, zh * S : (zh + 1) * S],
                                t8[a * 64 + zh * 32 : a * 64 + zh * 32 + 32, :],
                            )

                # ---- natural-layout transposes: 16*Qraw / 16*K2n [s, z2] ----
                qraw = sb2.tile([128, S], F16, tag="qraw")
                k2n = sb2.tile([128, S], F16, tag="k2n")
                for src, dst in ((qt2, qraw), (kt2, k2n)):
                    ps = ps_sc.tile([128, S], F16, tag="ps_sc", name=f"tr{p}_{dst.tensor.name}")
                    for i in range(NQ):
                        nc.tensor.transpose(
                            ps[:, i * 128 : (i + 1) * 128],
                            src[:, i * 128 : (i + 1) * 128],
                            ident_h[:],
                        )
                    nc.vector.tensor_scalar(dst[:], ps[:], float(SC_G), None, MUL)

                # previous pair's deferred dQ^T burst: emitted after this pair's
                # proj/transposes so the new scores/exps win scheduler priority
                emit_dq_burst()

                # ---- fused scores/exp/accumulate loop ------------------------
                zsum2 = sb2.tile([128, 16], F32, tag="zsum2")  # [(q), (head, qb)]
                dk_ps = [
                    ps_d.tile([128, S], F32, tag="ps_d", name=f"dk_ps{p}_{a}")
                    for a in range(2)
                ]
                PT_all = pp.tile([128, 2 * NQ * S], F16, tag="PT", bufs=1, name=f"PT{p}")
                qhr = qh[0:32, :].rearrange("p (zh s) -> p zh s", zh=2)
                khr = kh[0:32, :].rearrange("p (zh s) -> p zh s", zh=2)
                qhr2 = qh[32:64, :].rearrange("p (zh s) -> p zh s", zh=2)
                khr2 = kh[32:64, :].rearrange("p (zh s) -> p zh s", zh=2)
                for i in range(NQ):
                    # scores blocks [q_i, k] for both heads (fp8 DoubleRow)
                    pt_s = []
                    for a in range(2):
                        qa = qhr if a == 0 else qhr2
                        ka = khr if a == 0 else khr2
                        ps = ps_sc.tile([128, S], F32, tag="ps_sc", name=f"sc{p}_{i}_{a}")
                        for ch in range(NCH):
                            nc.tensor.matmul(
                                ps[:, ch * 512 : (ch + 1) * 512],
                                lhsT=qa[:, :, i * 128 : (i + 1) * 128],
                                rhs=ka[:, :, ch * 512 : (ch + 1) * 512],
                                start=True,
                                stop=True,
                                perf_mode=DR,
                            )
                        pt_s.append(ps)
                    # P blocks + Z row-sums
                    P_t = []
                    for a in range(2):
                        pb = pp.tile([128, S], F16, tag="P", name=f"P{p}_{i}_{a}")
                        nc.scalar.activation(
                            pb[:],
                            pt_s[a][:],
                            EXP,
                            scale=float(BETA / (SC_Q * SC_Q)),
                            accum_out=zsum2[:, a * NQ + i : a * NQ + i + 1],
                        )
                        P_t.append(pb)
                    # Qn block = 16*Qraw_i / Z_i
                    q2n_t = sb4.tile([128, 128], F16, tag="q2n", name=f"q2n{p}_{i}")
                    for a in range(2):
                        zq = sb4.tile([128, 1], F32, tag="zq", name=f"zq{p}_{i}_{a}")
                        nc.vector.reciprocal(zq[:], zsum2[:, a * NQ + i : a * NQ + i + 1])
                        nc.vector.tensor_scalar_mul(
                            q2n_t[:, a * 64 : (a + 1) * 64],
                            qraw[:, i * 128 + a * 64 : i * 128 + (a + 1) * 64],
                            zq[:],
                        )
                    # dK^T += Qn_i^T P_i (col-tiled pair; each head's
                    # accumulator owns its own psum tile/banks)
                    for a in range(2):
                        for ch in range(NCH):
                            nc.tensor.matmul(
                                dk_ps[a][a * 64 : (a + 1) * 64, ch * 512 : (ch + 1) * 512],
                                lhsT=q2n_t[:, a * 64 : (a + 1) * 64],
                                rhs=P_t[a][:, ch * 512 : (ch + 1) * 512],
                                start=(i == 0),
                                stop=(i == NQ - 1),
                            )
                    # scoresT blocks [k_i, q] (fp8 DoubleRow) and PT
                    st_s = []
                    for a in range(2):
                        qa = qhr if a == 0 else qhr2
                        ka = khr if a == 0 else khr2
                        ps = ps_sc.tile([128, S], F32, tag="ps_sc", name=f"st{p}_{i}_{a}")
                        for ch in range(NCH):
                            nc.tensor.matmul(
                                ps[:, ch * 512 : (ch + 1) * 512],
                                lhsT=ka[:, :, i * 128 : (i + 1) * 128],
                                rhs=qa[:, :, ch * 512 : (ch + 1) * 512],
                                start=True,
                                stop=True,
                                perf_mode=DR,
                            )
                        st_s.append(ps)
                    for a in range(2):
                        j = a * NQ + i
                        nc.scalar.activation(
                            PT_all[:, j * S : (j + 1) * S],
                            st_s[a][:],
                            EXP,
                            scale=float(BETA / (SC_Q * SC_Q)),
                        )

                # ---- Z^-1 broadcast [z2, q] then evacuate accumulators -------
                zinv2 = sb2.tile([128, 16], F32, tag="zinv2")
                nc.vector.reciprocal(zinv2[:], zsum2[:])
                zt_ps = ps_sc.tile([128, S], F32, tag="ps_sc", name=f"ztp{p}")
                nc.tensor.transpose(zt_ps[0:16, 0:128], zinv2[:], ident[:])
                nc.vector.tensor_copy(ztsb[:], zt_ps[0:16, 0:128])
                nc.sync.dma_start(
                    zrowA[:].rearrange("p (b c) -> p b c", b=NQ), ztsb[0:NQ, :]
                )
                nc.sync.dma_start(
                    zrowB[:].rearrange("p (b c) -> p b c", b=NQ), ztsb[NQ : 2 * NQ, :]
                )
                # partition_broadcast is only correct to base partition 0 ->
                # broadcast each head's Z row across a full tile, read halves.
                zbcA = sb2.tile([128, S], F32, tag="zbcA")
                zbcB = sb2.tile([128, S], F32, tag="zbcB")
                nc.gpsimd.partition_broadcast(zbcA[:], zrowA[:])
                nc.gpsimd.partition_broadcast(zbcB[:], zrowB[:])

                for a in range(2):
                    nc.vector.tensor_copy(
                        dkt2[a * 64 : (a + 1) * 64, p * S : (p + 1) * S],
                        dk_ps[a][a * 64 : (a + 1) * 64, :],
                    )

                # (the dQ^T burst for this pair is emitted lazily -- see
                # emit_dq_burst -- so the next pair's scores/exps get priority)
                pending_dq.append((PT_all, k2n, (zbcA, zbcB), p))

            emit_dq_burst()

            # ---- output projection  16 * sum_h dQ wq + dK wk -----------------
            for sb in range(NQ):
                ps = ps_sc.tile([128, S], F32, tag="ps_sc", name=f"op{sb}")
                for ch in range(NCH):
                    n = 0
                    for pq in range(NPAIR):
                        for dmat, wmat in ((dqt2, wnq), (dkt2, wnk)):
                            nc.tensor.matmul(
                                ps[:, ch * 512 : (ch + 1) * 512],
                                lhsT=dmat[:, pq * S + sb * 128 : pq * S + (sb + 1) * 128],
                                rhs=wmat[:, pq * D + ch * 512 : pq * D + ch * 512 + 512],
                                start=(n == 0),
                                stop=(n == 2 * NPAIR - 1),
                            )
                            n += 1
                go = sb4.tile([128, S], F16, tag="go", name=f"go{sb}")
                nc.vector.tensor_copy(go[:], ps[:])
                nc.sync.dma_start(gout[sb * 128 : (sb + 1) * 128, :], go[:])

    nc.compile()
    return nc


def core_inputs(x, wq, wk, core):
    """Per-core input arrays (host-side shard/layout prep)."""
    b = core // 4
    h0 = 4 * (core % 4)
    xT = np.ascontiguousarray(x[b].T).astype(np.float16)
    wqT2 = np.empty((NPAIR * D, 128), np.float16)
    wkT2 = np.empty((NPAIR * D, 128), np.float16)
    wq2n = np.empty((NPAIR * 128, D), np.float16)
    wk2n = np.empty((NPAIR * 128, D), np.float16)
    for p in range(NPAIR):
        ha, hb = h0 + 2 * p, h0 + 2 * p + 1
        wqT2[p * D : (p + 1) * D, 0:64] = wq[ha].T
        wqT2[p * D : (p + 1) * D, 64:128] = wq[hb].T
        wkT2[p * D : (p + 1) * D, 0:64] = wk[ha].T
        wkT2[p * D : (p + 1) * D, 64:128] = wk[hb].T
        wq2n[p * 128 : p * 128 + 64] = wq[ha]
        wq2n[p * 128 + 64 : (p + 1) * 128] = wq[hb]
        wk2n[p * 128 : p * 128 + 64] = wk[ha]
        wk2n[p * 128 + 64 : (p + 1) * 128] = wk[hb]
    return {"xT": xT, "wqT2": wqT2, "wkT2": wkT2, "wq2n": wq2n, "wk2n": wk2n}


def combine(gouts):
    """Host unshard: unscale, negate, all-reduce over head groups, add pos."""
    pos = np.linspace(-0.5, 0.5, S, dtype=np.float32)[:, None] * np.float32(POS_SCALE)
    out = np.empty((B, S, D), np.float32)
    for b in range(B):
        acc = np.zeros((S, D), np.float64)
        for c in range(4 * b, 4 * b + 4):
            acc += np.asarray(gouts[c], np.float64)
        out[b] = (pos.astype(np.float64) - acc / SC_G).astype(np.float32)
    return out


def kernel(x, wq, wk, trace=False):
    x = np.asarray(x, np.float32)
    wq = np.asarray(wq, np.float32)
    wk = np.asarray(wk, np.float32)
    if "nc" not in _CACHE:
        _CACHE["nc"] = build_nc()
    nc = _CACHE["nc"]
    from concourse.bass_utils import run_bass_kernel_spmd

    in_maps = [core_inputs(x, wq, wk, c) for c in range(N_CORES)]
    res = run_bass_kernel_spmd(nc, in_maps, core_ids=list(range(N_CORES)), trace=trace)
    _CACHE["last_result"] = res
    gouts = [r["gout"] for r in res.results]
    return combine(gouts)
